# revision 11
# baseline (speedup 1.0000x reference)
"""Trainium2 Bass kernel for the CNN_PHMM_VAE loss (profile-HMM forward + KLD).

Strategy: pure data parallel over batch (512 -> 8 cores x 64). Each core runs
the 256-column HMM forward recurrence in linear space over a [64 batch
partitions, 129 motif positions] state, with per-batch prefix-product
reparametrizations so each column costs 8 DVE instructions:

  op1: u1 = cI_row * FI'              op5: FM[1:] = t2 * EMcol  (+accum ZM)
  op2: t  = u1 + FM'                  op6: u3 = (FI' * rho) * grow
  op3: u2 = cD_row * FD'              op7: FI = (FM' * rho) + u3 (+accum ZI)
  op4: t2 = t + u2                    op8: FD[1:] = scan(q*state + FM[k-1])

State units (host precompute, fp64):
  FM~[k] = FM[k]/PM[k], PM[k] = prod_{j<k} e^{a[j,M2M]}
  FI^[k] = FI[k]/(sM2I4[k] PM[k]),  FD^[k] = FD[k]/(Dhat[k] PM[k])
  Dhat[k] = sM2D[k-1]/sM2M[k-1]
Emissions are gathered/exp'd/column-max-normalized on the host; the rho
scalar slot of ops 6/7 keeps the normalization uniform across states.
Numeric range is handled by a per-batch anchor C_b (from the static
drawup of the delete-chain coefficient walk) plus a runtime rescale of
the state every RS columns (folded into the scalar slots of ops 5/6/7 of
the following column). The host applies all log-scale corrections and
takes the final mean; KLD is computed on-device.
"""
import sys

sys.path.insert(0, "/opt/trn_rl_repo")

import os

import numpy as np

B, L, K, E = 512, 256, 128, 16
L = int(os.environ.get("PHMM_L", L))  # internal: small-L perf probes only
REPEAT = int(os.environ.get("PHMM_REPEAT", 1))  # internal: perf probes only
NCORES = 8
BS = B // NCORES
Kp1 = K + 1
RS = 4
NEV = L // RS - 1           # rescale events at l = RS, 2RS, ..., L-RS
NCHUNK = 8
CCOLS = L // NCHUNK

M2M, M2I, M2D, I2M, I2I, D2M, D2D = 0, 1, 2, 3, 4, 5, 6

_cache = {}


def _build_program():
    import concourse.bacc as bacc
    import concourse.tile as tile
    from concourse import mybir

    f32 = mybir.dt.float32
    Alu = mybir.AluOpType

    nc = bacc.Bacc("TRN2", target_bir_lowering=False, debug=False)

    em_d = nc.declare_dram_parameter("em", [BS, L * K], f32, isOutput=False)
    ci_d = nc.declare_dram_parameter("ci", [BS, K], f32, isOutput=False)
    cd_d = nc.declare_dram_parameter("cd", [BS, K], f32, isOutput=False)
    gr_d = nc.declare_dram_parameter("gr", [BS, Kp1], f32, isOutput=False)
    q_d = nc.declare_dram_parameter("q", [BS, Kp1], f32, isOutput=False)
    rho_d = nc.declare_dram_parameter("rho", [BS, L], f32, isOutput=False)
    fm0_d = nc.declare_dram_parameter("fm0", [BS, Kp1], f32, isOutput=False)
    icb_d = nc.declare_dram_parameter("icb", [BS, 1], f32, isOutput=False)
    al_d = nc.declare_dram_parameter("al", [BS, 3], f32, isOutput=False)
    mus_d = nc.declare_dram_parameter("mus", [BS, E], f32, isOutput=False)
    lv_d = nc.declare_dram_parameter("lv", [BS, E], f32, isOutput=False)
    v_d = nc.declare_dram_parameter("outv", [BS, 1], f32, isOutput=True)
    z_d = nc.declare_dram_parameter("outz", [BS, max(NEV, 1)], f32, isOutput=True)
    kld_d = nc.declare_dram_parameter("outk", [BS, 1], f32, isOutput=True)

    with tile.TileContext(nc) as tc:
        with tc.tile_pool(name="p", bufs=1) as pool:
            def T(shape, tag):
                return pool.tile(shape, f32, tag=tag, name=tag)

            em = [T([BS, CCOLS * K], f"em{j}") for j in range(NCHUNK)]
            ci = T([BS, K], "ci"); cd = T([BS, K], "cd")
            gr = T([BS, Kp1], "gr"); q = T([BS, Kp1], "q")
            rho = T([BS, L], "rho"); icb = T([BS, 1], "icb")
            al = T([BS, 3], "al")
            mus_t = T([BS, E], "mus"); lv_t = T([BS, E], "lv")

            fm0 = T([BS, Kp1], "fm0"); fi0 = T([BS, Kp1], "fi0")
            fd0 = T([BS, Kp1], "fd0")
            fm_ab = [T([BS, Kp1], "fma"), T([BS, Kp1], "fmb")]
            fi_ab = [T([BS, Kp1], "fia"), T([BS, Kp1], "fib")]
            fd_ab = [T([BS, Kp1], "fda"), T([BS, Kp1], "fdb")]
            u1_t = T([BS, K], "u1"); t_t = T([BS, K], "t")
            u2_t = T([BS, K], "u2"); t2_t = T([BS, K], "t2")
            u3_t = T([BS, Kp1], "u3")
            zm = T([BS, L], "zm"); zi = T([BS, L], "zi")
            zbuf = T([BS, max(NEV, 1)], "zbuf")
            zc_t = T([BS, 1], "zc"); r_t = T([BS, 1], "r")
            rr_t = T([BS, 1], "rr")
            w0_t = T([BS, 1], "w0"); w1_t = T([BS, 1], "w1")
            v_t = T([BS, 1], "v")
            m2_t = T([BS, E], "m2"); s1_t = T([BS, E], "s1")
            ee_t = T([BS, E], "ee"); s2_t = T([BS, E], "s2")
            red_t = T([BS, 1], "red"); kld_t = T([BS, 1], "kld")

            for j in range(NCHUNK):
                nc.sync.dma_start(em[j][:], em_d[:, j * CCOLS * K:(j + 1) * CCOLS * K])
            nc.sync.dma_start(ci[:], ci_d[:]); nc.sync.dma_start(cd[:], cd_d[:])
            nc.sync.dma_start(gr[:], gr_d[:]); nc.sync.dma_start(q[:], q_d[:])
            nc.sync.dma_start(rho[:], rho_d[:]); nc.sync.dma_start(icb[:], icb_d[:])
            nc.sync.dma_start(al[:], al_d[:]); nc.sync.dma_start(fm0[:], fm0_d[:])
            nc.sync.dma_start(mus_t[:], mus_d[:]); nc.sync.dma_start(lv_t[:], lv_d[:])

            nc.vector.memset(fi0[:], 0.0)
            nc.vector.memset(fd0[:], 0.0)
            for tl in fm_ab + fd_ab:
                nc.vector.memset(tl[:], 0.0)

            # FD0 = scan over FM0
            nc.vector.tensor_tensor_scan(
                out=fd0[:, 1:Kp1], data0=q[:, 1:Kp1], data1=fm0[:, 0:K],
                initial=0.0, op0=Alu.mult, op1=Alu.add)

            fm_p, fi_p, fd_p = fm0, fi0, fd0
            post_event = False
            ev = 0
            for l0 in range(1, REPEAT * L + 1):
                l = (l0 - 1) % L + 1
                if l == 1:
                    fm_p, fi_p, fd_p = fm0, fi0, fd0
                    ev = 0
                    post_event = False
                fm_n = fm_ab[l % 2]; fi_n = fi_ab[l % 2]; fd_n = fd_ab[l % 2]
                j, c = (l - 1) // CCOLS, (l - 1) % CCOLS
                emsl = em[j][:, c * K:(c + 1) * K]
                nc.vector.tensor_tensor(u1_t[:], ci[:], fi_p[:, 0:K], Alu.mult)
                nc.vector.tensor_tensor(t_t[:], u1_t[:], fm_p[:, 0:K], Alu.add)
                nc.vector.tensor_tensor(u2_t[:], cd[:], fd_p[:, 0:K], Alu.mult)
                nc.vector.tensor_tensor(t2_t[:], t_t[:], u2_t[:], Alu.add)
                sc5 = r_t[:] if post_event else 1.0
                sc67 = rr_t[:] if post_event else rho[:, l - 1:l]
                nc.vector.scalar_tensor_tensor(
                    out=fm_n[:, 1:Kp1], in0=t2_t[:], scalar=sc5, in1=emsl,
                    op0=Alu.mult, op1=Alu.mult, accum_out=zm[:, l - 1:l])
                nc.vector.scalar_tensor_tensor(
                    out=u3_t[:], in0=fi_p[:], scalar=sc67, in1=gr[:],
                    op0=Alu.mult, op1=Alu.mult)
                nc.vector.scalar_tensor_tensor(
                    out=fi_n[:], in0=fm_p[:], scalar=sc67, in1=u3_t[:],
                    op0=Alu.mult, op1=Alu.add, accum_out=zi[:, l - 1:l])
                nc.vector.tensor_tensor_scan(
                    out=fd_n[:, 1:Kp1], data0=q[:, 1:Kp1], data1=fm_n[:, 0:K],
                    initial=0.0, op0=Alu.mult, op1=Alu.add)
                post_event = False
                if l % RS == 0 and l < L:
                    # zbuf[ev] = ZM + ZI; r = 1/(zbuf[ev]*invCb); rr = r*rho[l]
                    nc.vector.tensor_tensor(
                        zbuf[:, ev:ev + 1], zm[:, l - 1:l], zi[:, l - 1:l], Alu.add)
                    nc.vector.scalar_tensor_tensor(
                        out=zc_t[:], in0=zbuf[:, ev:ev + 1], scalar=icb[:],
                        in1=zbuf[:, ev:ev + 1], op0=Alu.mult, op1=Alu.bypass)
                    nc.vector.reciprocal(r_t[:], zc_t[:])
                    nc.vector.tensor_tensor(rr_t[:], r_t[:], rho[:, l:l + 1], Alu.mult)
                    post_event = True
                    ev += 1
                fm_p, fi_p, fd_p = fm_n, fi_n, fd_n

            # readout v = aM*FM[K] + aI*FI[K] + aD*FD[K]
            nc.vector.scalar_tensor_tensor(
                out=w0_t[:], in0=fm_p[:, K:Kp1], scalar=al[:, 0:1],
                in1=fm_p[:, K:Kp1], op0=Alu.mult, op1=Alu.bypass)
            nc.vector.scalar_tensor_tensor(
                out=w1_t[:], in0=fi_p[:, K:Kp1], scalar=al[:, 1:2],
                in1=w0_t[:], op0=Alu.mult, op1=Alu.add)
            nc.vector.scalar_tensor_tensor(
                out=v_t[:], in0=fd_p[:, K:Kp1], scalar=al[:, 2:3],
                in1=w1_t[:], op0=Alu.mult, op1=Alu.add)
            nc.sync.dma_start(v_d[:], v_t[:])
            nc.sync.dma_start(z_d[:], zbuf[:])

            # KLD = -0.5 * sum(1 + lv - mus^2 - exp(lv))
            nc.vector.tensor_tensor(m2_t[:], mus_t[:], mus_t[:], Alu.mult)
            nc.vector.tensor_tensor(s1_t[:], lv_t[:], m2_t[:], Alu.subtract)
            nc.scalar.activation(ee_t[:], lv_t[:], mybir.ActivationFunctionType.Exp)
            nc.vector.tensor_tensor(s2_t[:], s1_t[:], ee_t[:], Alu.subtract)
            nc.vector.tensor_reduce(
                red_t[:], s2_t[:], axis=mybir.AxisListType.X, op=Alu.add)
            nc.scalar.activation(
                kld_t[:], red_t[:], mybir.ActivationFunctionType.Copy,
                bias=-0.5 * E, scale=-0.5)
            nc.sync.dma_start(kld_d[:], kld_t[:])

    nc.compile()
    return nc


def _precompute(batch_input, a, e_m):
    """Host precompute in fp64. Returns device tables + host corrections."""
    a = a.astype(np.float64)
    sM2M = np.exp(a[:, :, M2M]); sI2M = np.exp(a[:, :, I2M])
    sD2M = np.exp(a[:, :, D2M]); sM2I4 = 0.25 * np.exp(a[:, :, M2I])
    sI2I4 = 0.25 * np.exp(a[:, :, I2I]); sM2D = np.exp(a[:, :, M2D])
    sD2D = np.exp(a[:, :, D2D])
    Bn = a.shape[0]

    Dhat = np.ones((Bn, Kp1))
    Dhat[:, 1:] = sM2D[:, :-1] / sM2M[:, :-1]
    cI = (sI2M * sM2I4 / sM2M)[:, :K]
    cD = (sD2M * Dhat / sM2M)[:, :K]
    grow = sI2I4
    lq = np.zeros((Bn, Kp1))
    lq[:, 1:] = (a[:, :-1, D2D] + np.log(Dhat[:, :-1]) - np.log(Dhat[:, 1:])
                 - a[:, :-1, M2M])
    q = np.exp(lq); q[:, 0] = 0.0

    # per-batch anchor from the max drawup of the q-prefix walk
    pref = np.cumsum(lq, axis=1)
    runmin = np.minimum.accumulate(pref, axis=1)
    Qspread = np.max(pref - runmin, axis=1)
    lcD = np.log(cD).max(axis=1)
    headD = Qspread + np.maximum(lcD, 0.0)
    logCb = np.minimum(45.0, 88.0 - 25.0 - headD)
    Cb = np.exp(logCb)

    logPMK = a[:, :K, M2M].sum(axis=1)
    alphas = np.stack([sM2M[:, K], sI2M[:, K] * sM2I4[:, K],
                       sD2M[:, K] * Dhat[:, K]], axis=1)

    bi = np.arange(Bn)[:, None, None]
    ki = np.arange(K)[None, None, :]
    EM = np.exp(e_m.astype(np.float64)[bi, ki, batch_input[:, :, None]])  # (B,L,K)
    Mn = EM.max(axis=2)
    EM = EM / Mn[:, :, None]
    rho = 1.0 / Mn
    logMnorm = np.log(Mn).sum(axis=1)

    fm0 = np.zeros((Bn, Kp1))
    fm0[:, 0] = Cb

    f = np.float32
    tables = dict(
        em=EM.reshape(Bn, L * K).astype(f), ci=cI.astype(f), cd=cD.astype(f),
        gr=grow.astype(f), q=q.astype(f), rho=rho.astype(f),
        fm0=fm0.astype(f), icb=(1.0 / Cb)[:, None].astype(f),
        al=alphas.astype(f),
    )
    corr = dict(logCb=logCb, logMnorm=logMnorm, logPMK=logPMK)
    return tables, corr


def _get_exec():
    """Build program + a cached jitted shard_map executor (one compile)."""
    if "exec" in _cache:
        return _cache["exec"]
    import jax
    from jax.sharding import Mesh, PartitionSpec
    from jax.experimental.shard_map import shard_map
    from concourse import mybir
    from concourse.bass2jax import (
        install_neuronx_cc_hook, _bass_exec_p, partition_id_tensor)

    nc = _build_program()
    install_neuronx_cc_hook()

    pname = nc.partition_id_tensor.name if nc.partition_id_tensor else None
    in_names, out_names, out_avals, zero_shapes = [], [], [], []
    for alloc in nc.m.functions[0].allocations:
        if not isinstance(alloc, mybir.MemoryLocationSet):
            continue
        name = alloc.memorylocations[0].name
        if alloc.kind == "ExternalInput":
            if name != pname:
                in_names.append(name)
        elif alloc.kind == "ExternalOutput":
            shape = tuple(alloc.tensor_shape)
            dtype = mybir.dt.np(alloc.dtype)
            out_names.append(name)
            out_avals.append(jax.core.ShapedArray(shape, dtype))
            zero_shapes.append((shape, dtype))
    n_params = len(in_names)
    all_names = in_names + out_names
    if pname is not None:
        all_names = all_names + [pname]
    donate = tuple(range(n_params, n_params + len(out_names)))

    def _body(*args):
        operands = list(args)
        if pname is not None:
            operands.append(partition_id_tensor())
        outs = _bass_exec_p.bind(
            *operands, out_avals=tuple(out_avals), in_names=tuple(all_names),
            out_names=tuple(out_names), lowering_input_output_aliases=(),
            sim_require_finite=True, sim_require_nnan=True, nc=nc)
        return tuple(outs)

    devices = jax.devices()[:NCORES]
    mesh = Mesh(np.asarray(devices), ("core",))
    in_specs = (PartitionSpec("core"),) * (n_params + len(out_names))
    out_specs = (PartitionSpec("core"),) * len(out_names)
    sharded = jax.jit(
        shard_map(_body, mesh=mesh, in_specs=in_specs, out_specs=out_specs,
                  check_rep=False),
        donate_argnums=donate, keep_unused=True)
    _cache["exec"] = (sharded, in_names, out_names, out_avals, n_params)
    return _cache["exec"]


def _run_device(tables_full):
    """tables_full: dict name -> full [B, ...] array. Returns dict of outputs
    concatenated over cores as [B, ...]."""
    sharded, in_names, out_names, out_avals, n_params = _get_exec()
    ins = [np.ascontiguousarray(tables_full[n]) for n in in_names]
    zeros = [np.zeros((NCORES * a.shape[0], *a.shape[1:]), a.dtype)
             for a in out_avals]
    outs = sharded(*ins, *zeros)
    return {n: np.asarray(o) for n, o in zip(out_names, outs)}


def kernel(batch_input, transition_probs, emission_probs, mus, logvars):
    batch_input = np.asarray(batch_input).astype(np.int64)
    a = np.asarray(transition_probs, dtype=np.float32)
    e_m = np.asarray(emission_probs, dtype=np.float32)
    mus = np.asarray(mus, dtype=np.float32)
    logvars = np.asarray(logvars, dtype=np.float32)

    tables, corr = _precompute(batch_input, a, e_m)
    tables["mus"] = mus
    tables["lv"] = logvars

    out = _run_device(tables)
    v = out["outv"][:, 0]
    z = out["outz"]
    kld = out["outk"][:, 0]

    v64 = np.maximum(v.astype(np.float64), 1e-300)
    z64 = np.maximum(z.astype(np.float64), 1e-300)
    logCb = corr["logCb"]
    nll = -(np.log(v64) - logCb + (np.log(z64) - logCb[:, None]).sum(axis=1)
            + corr["logMnorm"] + corr["logPMK"])
    loss = nll.mean() + kld.astype(np.float64).mean()
    return np.float32(loss)


# revision 14
# speedup vs baseline: 1.2006x; 1.2006x over previous
"""Trainium2 Bass kernel for the CNN_PHMM_VAE loss (profile-HMM forward + KLD).

Strategy: pure data parallel over batch (512 -> 8 cores x 64). Each core runs
the 256-column HMM forward recurrence in linear space over a [64 batch
partitions, 129 motif positions] state, with per-batch prefix-product
reparametrizations so each column costs 8 DVE instructions:

  op1: u1 = cI_row * FI'              op5: FM[1:] = t2 * EMcol  (+accum ZM)
  op2: t  = u1 + FM'                  op6: u3 = (FI' * rho) * grow
  op3: u2 = cD_row * FD'              op7: FI = (FM' * rho) + u3 (+accum ZI)
  op4: t2 = t + u2                    op8: FD[1:] = scan(q*state + FM[k-1])

State units (host precompute, fp64):
  FM~[k] = FM[k]/PM[k], PM[k] = prod_{j<k} e^{a[j,M2M]}
  FI^[k] = FI[k]/(sM2I4[k] PM[k]),  FD^[k] = FD[k]/(Dhat[k] PM[k])
  Dhat[k] = sM2D[k-1]/sM2M[k-1]
Emissions are gathered/exp'd/column-max-normalized on the host; the rho
scalar slot of ops 6/7 keeps the normalization uniform across states.
Numeric range is handled by a per-batch anchor C_b (from the static
drawup of the delete-chain coefficient walk) plus a runtime rescale of
the state every RS columns (folded into the scalar slots of ops 5/6/7 of
the following column). The host applies all log-scale corrections and
takes the final mean; KLD is computed on-device.
"""
import sys

sys.path.insert(0, "/opt/trn_rl_repo")

import os

import numpy as np

B, L, K, E = 512, 256, 128, 16
L = int(os.environ.get("PHMM_L", L))  # internal: small-L perf probes only
REPEAT = int(os.environ.get("PHMM_REPEAT", 1))  # internal: perf probes only
NCORES = 8
BS = B // NCORES
Kp1 = K + 1
RS = 4
NEV = L // RS - 1           # rescale events at l = RS, 2RS, ..., L-RS
NCHUNK = 8
CCOLS = L // NCHUNK

M2M, M2I, M2D, I2M, I2I, D2M, D2D = 0, 1, 2, 3, 4, 5, 6

_cache = {}


def _build_program():
    import concourse.bacc as bacc
    import concourse.tile as tile
    from concourse import mybir

    f32 = mybir.dt.float32
    Alu = mybir.AluOpType

    nc = bacc.Bacc("TRN2", target_bir_lowering=False, debug=False)

    em_d = nc.declare_dram_parameter("em", [BS, L * K], f32, isOutput=False)
    ci_d = nc.declare_dram_parameter("ci", [BS, K], f32, isOutput=False)
    cd_d = nc.declare_dram_parameter("cd", [BS, K], f32, isOutput=False)
    gr_d = nc.declare_dram_parameter("gr", [BS, Kp1], f32, isOutput=False)
    q_d = nc.declare_dram_parameter("q", [BS, Kp1], f32, isOutput=False)
    rho_d = nc.declare_dram_parameter("rho", [BS, L], f32, isOutput=False)
    fm0_d = nc.declare_dram_parameter("fm0", [BS, Kp1], f32, isOutput=False)
    icb_d = nc.declare_dram_parameter("icb", [BS, 1], f32, isOutput=False)
    al_d = nc.declare_dram_parameter("al", [BS, 3], f32, isOutput=False)
    mus_d = nc.declare_dram_parameter("mus", [BS, E], f32, isOutput=False)
    lv_d = nc.declare_dram_parameter("lv", [BS, E], f32, isOutput=False)
    v_d = nc.declare_dram_parameter("outv", [BS, 1], f32, isOutput=True)
    z_d = nc.declare_dram_parameter("outz", [BS, max(NEV, 1)], f32, isOutput=True)
    kld_d = nc.declare_dram_parameter("outk", [BS, 1], f32, isOutput=True)

    with tile.TileContext(nc) as tc:
        with tc.tile_pool(name="p", bufs=1) as pool:
            def T(shape, tag):
                return pool.tile(shape, f32, tag=tag, name=tag)

            em = [T([BS, CCOLS * K], f"em{j}") for j in range(NCHUNK)]
            ci = T([BS, K], "ci"); cd = T([BS, K], "cd")
            gr = T([BS, Kp1], "gr"); q = T([BS, Kp1], "q")
            rho = T([BS, L], "rho"); icb = T([BS, 1], "icb")
            al = T([BS, 3], "al")
            mus_t = T([BS, E], "mus"); lv_t = T([BS, E], "lv")

            fm0 = T([BS, Kp1], "fm0"); fi0 = T([BS, Kp1], "fi0")
            fd0 = T([BS, Kp1], "fd0")
            fm_ab = [T([BS, Kp1], "fma"), T([BS, Kp1], "fmb")]
            fi_ab = [T([BS, Kp1], "fia"), T([BS, Kp1], "fib")]
            fd_ab = [T([BS, Kp1], "fda"), T([BS, Kp1], "fdb")]
            u1_t = T([BS, K], "u1"); t_t = T([BS, K], "t")
            u2_t = T([BS, K], "u2"); t2_t = T([BS, K], "t2")
            u3_t = T([BS, Kp1], "u3")
            zm = T([BS, L], "zm"); zi = T([BS, L], "zi")
            zbuf = T([BS, max(NEV, 1)], "zbuf")
            zc_t = T([BS, 1], "zc"); r_t = T([BS, 1], "r")
            rr_t = T([BS, 1], "rr")
            w0_t = T([BS, 1], "w0"); w1_t = T([BS, 1], "w1")
            v_t = T([BS, 1], "v")
            m2_t = T([BS, E], "m2"); s1_t = T([BS, E], "s1")
            ee_t = T([BS, E], "ee"); s2_t = T([BS, E], "s2")
            red_t = T([BS, 1], "red"); kld_t = T([BS, 1], "kld")

            for j in range(NCHUNK):
                nc.sync.dma_start(em[j][:], em_d[:, j * CCOLS * K:(j + 1) * CCOLS * K])
            nc.sync.dma_start(ci[:], ci_d[:]); nc.sync.dma_start(cd[:], cd_d[:])
            nc.sync.dma_start(gr[:], gr_d[:]); nc.sync.dma_start(q[:], q_d[:])
            nc.sync.dma_start(rho[:], rho_d[:]); nc.sync.dma_start(icb[:], icb_d[:])
            nc.sync.dma_start(al[:], al_d[:]); nc.sync.dma_start(fm0[:], fm0_d[:])
            nc.sync.dma_start(mus_t[:], mus_d[:]); nc.sync.dma_start(lv_t[:], lv_d[:])

            nc.vector.memset(fi0[:], 0.0)
            nc.vector.memset(fd0[:], 0.0)
            for tl in fm_ab + fd_ab:
                nc.vector.memset(tl[:], 0.0)

            # FD0 = scan over FM0
            nc.vector.tensor_tensor_scan(
                out=fd0[:, 1:Kp1], data0=q[:, 1:Kp1], data1=fm0[:, 0:K],
                initial=0.0, op0=Alu.mult, op1=Alu.add)

            fm_p, fi_p, fd_p = fm0, fi0, fd0
            post_event = False
            ev = 0
            for l0 in range(1, REPEAT * L + 1):
                l = (l0 - 1) % L + 1
                if l == 1:
                    fm_p, fi_p, fd_p = fm0, fi0, fd0
                    ev = 0
                    post_event = False
                fm_n = fm_ab[l % 2]; fi_n = fi_ab[l % 2]; fd_n = fd_ab[l % 2]
                j, c = (l - 1) // CCOLS, (l - 1) % CCOLS
                emsl = em[j][:, c * K:(c + 1) * K]
                nc.vector.tensor_tensor(u1_t[:], ci[:], fi_p[:, 0:K], Alu.mult)
                nc.vector.tensor_tensor(t_t[:], u1_t[:], fm_p[:, 0:K], Alu.add)
                nc.vector.tensor_tensor(u2_t[:], cd[:], fd_p[:, 0:K], Alu.mult)
                nc.vector.tensor_tensor(t2_t[:], t_t[:], u2_t[:], Alu.add)
                sc5 = r_t[:] if post_event else 1.0
                sc67 = rr_t[:] if post_event else rho[:, l - 1:l]
                nc.vector.scalar_tensor_tensor(
                    out=fm_n[:, 1:Kp1], in0=t2_t[:], scalar=sc5, in1=emsl,
                    op0=Alu.mult, op1=Alu.mult, accum_out=zm[:, l - 1:l])
                nc.vector.scalar_tensor_tensor(
                    out=u3_t[:], in0=fi_p[:], scalar=sc67, in1=gr[:],
                    op0=Alu.mult, op1=Alu.mult)
                nc.vector.scalar_tensor_tensor(
                    out=fi_n[:], in0=fm_p[:], scalar=sc67, in1=u3_t[:],
                    op0=Alu.mult, op1=Alu.add, accum_out=zi[:, l - 1:l])
                nc.vector.tensor_tensor_scan(
                    out=fd_n[:, 1:Kp1], data0=q[:, 1:Kp1], data1=fm_n[:, 0:K],
                    initial=0.0, op0=Alu.mult, op1=Alu.add)
                post_event = False
                if l % RS == 0 and l < L:
                    # zbuf[ev] = ZM + ZI; r = 1/(zbuf[ev]*invCb); rr = r*rho[l]
                    # Scalar-engine ops keep the event chain off the busy DVE;
                    # only the reciprocal (DVE-only op) stays there.
                    nc.scalar.activation(
                        zbuf[:, ev:ev + 1], zm[:, l - 1:l],
                        mybir.ActivationFunctionType.Identity,
                        bias=zi[:, l - 1:l], scale=1.0)
                    nc.scalar.mul(zc_t[:], zbuf[:, ev:ev + 1], icb[:])
                    nc.vector.reciprocal(r_t[:], zc_t[:])
                    nc.scalar.mul(rr_t[:], rho[:, l:l + 1], r_t[:])
                    post_event = True
                    ev += 1
                fm_p, fi_p, fd_p = fm_n, fi_n, fd_n

            # readout v = aM*FM[K] + aI*FI[K] + aD*FD[K]
            nc.vector.scalar_tensor_tensor(
                out=w0_t[:], in0=fm_p[:, K:Kp1], scalar=al[:, 0:1],
                in1=fm_p[:, K:Kp1], op0=Alu.mult, op1=Alu.bypass)
            nc.vector.scalar_tensor_tensor(
                out=w1_t[:], in0=fi_p[:, K:Kp1], scalar=al[:, 1:2],
                in1=w0_t[:], op0=Alu.mult, op1=Alu.add)
            nc.vector.scalar_tensor_tensor(
                out=v_t[:], in0=fd_p[:, K:Kp1], scalar=al[:, 2:3],
                in1=w1_t[:], op0=Alu.mult, op1=Alu.add)
            nc.sync.dma_start(v_d[:], v_t[:])
            nc.sync.dma_start(z_d[:], zbuf[:])

            # KLD = -0.5 * sum(1 + lv - mus^2 - exp(lv))
            nc.vector.tensor_tensor(m2_t[:], mus_t[:], mus_t[:], Alu.mult)
            nc.vector.tensor_tensor(s1_t[:], lv_t[:], m2_t[:], Alu.subtract)
            nc.scalar.activation(ee_t[:], lv_t[:], mybir.ActivationFunctionType.Exp)
            nc.vector.tensor_tensor(s2_t[:], s1_t[:], ee_t[:], Alu.subtract)
            nc.vector.tensor_reduce(
                red_t[:], s2_t[:], axis=mybir.AxisListType.X, op=Alu.add)
            nc.scalar.activation(
                kld_t[:], red_t[:], mybir.ActivationFunctionType.Copy,
                bias=-0.5 * E, scale=-0.5)
            nc.sync.dma_start(kld_d[:], kld_t[:])

    nc.compile()
    return nc


def _precompute(batch_input, a, e_m):
    """Host precompute in fp64. Returns device tables + host corrections."""
    a = a.astype(np.float64)
    sM2M = np.exp(a[:, :, M2M]); sI2M = np.exp(a[:, :, I2M])
    sD2M = np.exp(a[:, :, D2M]); sM2I4 = 0.25 * np.exp(a[:, :, M2I])
    sI2I4 = 0.25 * np.exp(a[:, :, I2I]); sM2D = np.exp(a[:, :, M2D])
    sD2D = np.exp(a[:, :, D2D])
    Bn = a.shape[0]

    Dhat = np.ones((Bn, Kp1))
    Dhat[:, 1:] = sM2D[:, :-1] / sM2M[:, :-1]
    cI = (sI2M * sM2I4 / sM2M)[:, :K]
    cD = (sD2M * Dhat / sM2M)[:, :K]
    grow = sI2I4
    lq = np.zeros((Bn, Kp1))
    lq[:, 1:] = (a[:, :-1, D2D] + np.log(Dhat[:, :-1]) - np.log(Dhat[:, 1:])
                 - a[:, :-1, M2M])
    q = np.exp(lq); q[:, 0] = 0.0

    # per-batch anchor from the max drawup of the q-prefix walk
    pref = np.cumsum(lq, axis=1)
    runmin = np.minimum.accumulate(pref, axis=1)
    Qspread = np.max(pref - runmin, axis=1)
    lcD = np.log(cD).max(axis=1)
    headD = Qspread + np.maximum(lcD, 0.0)
    logCb = np.minimum(45.0, 88.0 - 25.0 - headD)
    Cb = np.exp(logCb)

    logPMK = a[:, :K, M2M].sum(axis=1)
    alphas = np.stack([sM2M[:, K], sI2M[:, K] * sM2I4[:, K],
                       sD2M[:, K] * Dhat[:, K]], axis=1)

    bi = np.arange(Bn)[:, None, None]
    ki = np.arange(K)[None, None, :]
    EM = np.exp(e_m.astype(np.float64)[bi, ki, batch_input[:, :, None]])  # (B,L,K)
    Mn = EM.max(axis=2)
    EM = EM / Mn[:, :, None]
    rho = 1.0 / Mn
    logMnorm = np.log(Mn).sum(axis=1)

    fm0 = np.zeros((Bn, Kp1))
    fm0[:, 0] = Cb

    f = np.float32
    tables = dict(
        em=EM.reshape(Bn, L * K).astype(f), ci=cI.astype(f), cd=cD.astype(f),
        gr=grow.astype(f), q=q.astype(f), rho=rho.astype(f),
        fm0=fm0.astype(f), icb=(1.0 / Cb)[:, None].astype(f),
        al=alphas.astype(f),
    )
    corr = dict(logCb=logCb, logMnorm=logMnorm, logPMK=logPMK)
    return tables, corr


def _get_exec():
    """Build program + a cached jitted shard_map executor (one compile)."""
    if "exec" in _cache:
        return _cache["exec"]
    import jax
    from jax.sharding import Mesh, PartitionSpec
    from jax.experimental.shard_map import shard_map
    from concourse import mybir
    from concourse.bass2jax import (
        install_neuronx_cc_hook, _bass_exec_p, partition_id_tensor)

    nc = _build_program()
    install_neuronx_cc_hook()

    pname = nc.partition_id_tensor.name if nc.partition_id_tensor else None
    in_names, out_names, out_avals, zero_shapes = [], [], [], []
    for alloc in nc.m.functions[0].allocations:
        if not isinstance(alloc, mybir.MemoryLocationSet):
            continue
        name = alloc.memorylocations[0].name
        if alloc.kind == "ExternalInput":
            if name != pname:
                in_names.append(name)
        elif alloc.kind == "ExternalOutput":
            shape = tuple(alloc.tensor_shape)
            dtype = mybir.dt.np(alloc.dtype)
            out_names.append(name)
            out_avals.append(jax.core.ShapedArray(shape, dtype))
            zero_shapes.append((shape, dtype))
    n_params = len(in_names)
    all_names = in_names + out_names
    if pname is not None:
        all_names = all_names + [pname]
    donate = tuple(range(n_params, n_params + len(out_names)))

    def _body(*args):
        operands = list(args)
        if pname is not None:
            operands.append(partition_id_tensor())
        outs = _bass_exec_p.bind(
            *operands, out_avals=tuple(out_avals), in_names=tuple(all_names),
            out_names=tuple(out_names), lowering_input_output_aliases=(),
            sim_require_finite=True, sim_require_nnan=True, nc=nc)
        return tuple(outs)

    devices = jax.devices()[:NCORES]
    mesh = Mesh(np.asarray(devices), ("core",))
    in_specs = (PartitionSpec("core"),) * (n_params + len(out_names))
    out_specs = (PartitionSpec("core"),) * len(out_names)
    sharded = jax.jit(
        shard_map(_body, mesh=mesh, in_specs=in_specs, out_specs=out_specs,
                  check_rep=False),
        donate_argnums=donate, keep_unused=True)
    _cache["exec"] = (sharded, in_names, out_names, out_avals, n_params)
    return _cache["exec"]


def _run_device(tables_full):
    """tables_full: dict name -> full [B, ...] array. Returns dict of outputs
    concatenated over cores as [B, ...]."""
    sharded, in_names, out_names, out_avals, n_params = _get_exec()
    ins = [np.ascontiguousarray(tables_full[n]) for n in in_names]
    zeros = [np.zeros((NCORES * a.shape[0], *a.shape[1:]), a.dtype)
             for a in out_avals]
    outs = sharded(*ins, *zeros)
    return {n: np.asarray(o) for n, o in zip(out_names, outs)}


def kernel(batch_input, transition_probs, emission_probs, mus, logvars):
    batch_input = np.asarray(batch_input).astype(np.int64)
    a = np.asarray(transition_probs, dtype=np.float32)
    e_m = np.asarray(emission_probs, dtype=np.float32)
    mus = np.asarray(mus, dtype=np.float32)
    logvars = np.asarray(logvars, dtype=np.float32)

    tables, corr = _precompute(batch_input, a, e_m)
    tables["mus"] = mus
    tables["lv"] = logvars

    out = _run_device(tables)
    v = out["outv"][:, 0]
    z = out["outz"]
    kld = out["outk"][:, 0]

    v64 = np.maximum(v.astype(np.float64), 1e-300)
    z64 = np.maximum(z.astype(np.float64), 1e-300)
    logCb = corr["logCb"]
    nll = -(np.log(v64) - logCb + (np.log(z64) - logCb[:, None]).sum(axis=1)
            + corr["logMnorm"] + corr["logPMK"])
    loss = nll.mean() + kld.astype(np.float64).mean()
    return np.float32(loss)
